# revision 4
# baseline (speedup 1.0000x reference)
"""Trainium2 Bass kernel for nn_AttentionBlock (B=4, S=2048, D=1024).

Sharding: 8 cores = 4 batches x 2 query-halves. Each core owns 1024
queries of one batch and produces y[own queries, 1024] directly in
row-major [q, f] layout. All matmuls bf16 (fp32 PSUM accumulate),
rel err ~2e-3 (gate 2e-2). HW exec ~188-190us (vs 292.8us baseline).

Host-side weight folds (x-independent):
    W2 = Wk^T @ Wq  ->  scoresT = X W2 Xq^T   ([key, query] layout)
    W3 = Wp @ Wv    ->  y = attn (X W3^T) = attn VP
Q, K, V, and the projection never exist on the device.

PE stream (~99% dense, 752 x 512-col matmuls ~= 169us):
  1a: G[0:4]  = W2[g,:] @ Xq^T                              (64 MMs)
  2a: VP = X @ W3^T for the core's OWN 1024 keys only      (128 MMs)
      -> DRAM bounce -> two pairwise AllGathers ([[0,1],[2,3],[4,5],
      [6,7]], 1MB each, first triggered after 4 VP tiles) assemble the
      full [2048, 1024] VP on both cores, overlapped with 1b+2b.
      Removes the VP duplication across the batch pair (~28us of PE).
      SPMD rank-symmetry: own keys == own queries (same global half),
      and the gathered buffers are in GLOBAL key order on both pair
      members, so the single shared graph stays rank-agnostic.
  1b: G[4:8]                                                (64 MMs)
  2b: scoresT[sk] = X^T[sk].T @ G; exp on ScalarE -> bf16 expT;
      fp32 acc_sb += expT on DVE                           (256 MMs)
  rowsum: 8 tiny N=1 matmuls (stationary acc chunk, moving ones col)
      -> [128q, 1] per query chunk; one DVE reciprocal [128, 8]
  4:  y[qc] = sum_sk expT[sk][:, qc].T @ VP[sk]            (256 MMs)
      epilogue: ScalarE Copy scaled by the per-partition reciprocal
      column, DVE add of a pre-broadcast bias tile, DMA out.

Scheduling notes (hard-won):
  - The Sync sequencer pushes dma_starts strictly in order at ~0.6us
    each; bulk pushes ahead of a latency-critical descriptor delay it
    by tens of us. So: inputs are coalesced to one push per [128, 1024]
    tile (the xtr host layout gives one push per KEY TILE), and all
    collective-path DMAs + triggers live on the idle GPSIMD queue.
  - The AllGather data phase occupies all 16 DMA engines (~12-45us,
    high variance); every input the compute needs is SBUF-resident
    before it starts, so nothing starves mid-collective.
  - Gather-ins are emitted in phase-4 consumption order (tiles 0-3
    after collective A, 4-7 after B, 8-15 instantly), so pass 0 can
    start while a slow second exchange is still in flight.
"""

import numpy as np
from contextlib import ExitStack

D = 1024
S = 2048
SQ = 1024  # queries per core
P = 128
ND = D // P   # 8
NS = S // P   # 16
SCALE = float(1.0 / np.sqrt(np.float32(D)).astype(np.float32))

_CACHED = {}


def _build_nc():
    import concourse.tile as tile
    from concourse import bacc, mybir

    BF = mybir.dt.bfloat16
    FP = mybir.dt.float32
    Exp = mybir.ActivationFunctionType.Exp
    Copy = mybir.ActivationFunctionType.Copy
    ADD = mybir.AluOpType.add

    nc = bacc.Bacc("TRN2", target_bir_lowering=False, num_devices=8)
    # xtr: host-rearranged X so each key tile sk is one contiguous row
    # block: xtr[sk*128 + p, d*128 + j] = X[sk*128 + j, d*128 + p].
    # One DMA push per key tile (16 total) instead of 128.
    xtr_d = nc.declare_dram_parameter("xtr", [S, D], BF, isOutput=False)
    xqt_d = nc.declare_dram_parameter("xqt", [D, SQ], BF, isOutput=False)
    w2t_d = nc.declare_dram_parameter("w2t", [D, D], BF, isOutput=False)
    w3t_d = nc.declare_dram_parameter("w3t", [D, D], BF, isOutput=False)
    biasb_d = nc.declare_dram_parameter("biasb", [P, D], FP, isOutput=False)
    onesc_d = nc.declare_dram_parameter("onesc", [P, 1], FP, isOutput=False)
    y_d = nc.declare_dram_parameter("y", [SQ, D], FP, isOutput=True)

    with tile.TileContext(nc) as tc:
        with ExitStack() as ctx:
            pool = ctx.enter_context(tc.tile_pool(name="main", bufs=1))
            psum = ctx.enter_context(tc.tile_pool(name="psum", bufs=1, space="PSUM"))
            dram = ctx.enter_context(tc.tile_pool(name="dram", bufs=1, space="DRAM"))

            def ptile(shape, name, tag, bufs=1, dt=BF):
                return pool.tile(shape, dt, name=name, tag=tag, bufs=bufs)

            def qbank(i, name, shape=(P, 1024)):
                return psum.tile(list(shape), FP, name=name, tag=f"q{i}", bufs=1)

            # ---- resident inputs ----
            xq = []
            w2t = []
            for d in range(ND):
                t = ptile([P, SQ], f"xq{d}", f"xq{d}")
                nc.sync.dma_start(t[:], xqt_d[d * P:(d + 1) * P, :])
                xq.append(t)
                t = ptile([P, D], f"w2t{d}", f"w2t{d}")
                nc.sync.dma_start(t[:], w2t_d[d * P:(d + 1) * P, :])
                w2t.append(t)
            w3t = []
            for d in range(ND):
                t = ptile([P, D], f"w3t{d}", f"w3t{d}")
                nc.sync.dma_start(t[:], w3t_d[d * P:(d + 1) * P, :])
                w3t.append(t)
            onesc_sb = ptile([P, 1], "onesc", "onesc", dt=FP)
            nc.sync.dma_start(onesc_sb[:], onesc_d[:, :])
            biasb_sb = ptile([P, D], "biasb", "biasb", dt=FP)
            nc.sync.dma_start(biasb_sb[:], biasb_d[:, :])
            # preload the full X^T key stream: all input DMA lands by ~31us,
            # well before the collective data phase hogs the DMA engines,
            # so the scores units never touch DMA mid-collective.
            xs_t = []
            for sk in range(NS):
                t = ptile([P, D], f"xs_{sk}", f"xs_{sk}")
                nc.sync.dma_start(t[:], xtr_d[sk * P:(sk + 1) * P, :])
                xs_t.append(t)

            # DRAM bounce buffers for the VP exchange, split in two halves:
            # each transfer is 1MB instead of 2MB and the first one starts
            # ~20us earlier, thinning the slow-collective tail
            own_vp_a = dram.tile([SQ // 2, D], BF)
            own_vp_b = dram.tile([SQ // 2, D], BF)
            gath_vp_a = dram.tile([S // 2, D], BF)
            gath_vp_b = dram.tile([S // 2, D], BF)

            # ---- phase 1: G[g][128, SQ] = sum_d W2[g,d] @ Xq^T[d] ----
            g_sb = []
            for g in range(ND):
                g_sb.append(ptile([P, SQ], f"g{g}", f"g{g}"))

            def g_unit(g):
                pg = qbank(g % 4, f"pg_{g}")
                for d in range(ND):
                    lt = w2t[d][:, g * P:(g + 1) * P]
                    nc.tensor.matmul(pg[:, 0:512], lt, xq[d][:, 0:512],
                                     start=(d == 0), stop=(d == ND - 1))
                    nc.tensor.matmul(pg[:, 512:1024], lt, xq[d][:, 512:1024],
                                     start=(d == 0), stop=(d == ND - 1))
                nc.vector.tensor_copy(g_sb[g][:], pg[:])

            # half of phase 1, then the VP units: the first collective
            # triggers ~35us earlier, capping slow-exchange stalls
            for g in range(4):
                g_unit(g)

            # ---- phase 2a: VP[j][128, 1024] = X_own[j-blk] @ W3^T for own
            #      keys only; bounce each tile to DRAM; pair-AllGather.
            #      All collective-path DMAs + the trigger live on the GPSIMD
            #      queue so they are not pushed behind the sync queue's
            #      ~160 input pushes.
            vp_own = []
            for j in range(ND):
                vp_own.append(ptile([P, D], f"vpo{j}", f"vpo{j % 4}", bufs=2))
            for j in range(ND):
                pvp = qbank(2 + j % 2, f"pv_{j}")
                for d in range(ND):
                    # own keys == own queries (same global half), so the
                    # stationary tile is a slice of the resident xq tiles
                    lt = xq[d][:, j * P:(j + 1) * P]
                    nc.tensor.matmul(pvp[:, 0:512], lt, w3t[d][:, 0:512],
                                     start=(d == 0), stop=(d == ND - 1))
                    nc.tensor.matmul(pvp[:, 512:1024], lt,
                                     w3t[d][:, 512:1024],
                                     start=(d == 0), stop=(d == ND - 1))
                nc.vector.tensor_copy(vp_own[j][:], pvp[:])
                half = own_vp_a if j < 4 else own_vp_b
                nc.gpsimd.dma_start(half[(j % 4) * P:(j % 4 + 1) * P, :],
                                    vp_own[j][:])
                if j == 3:
                    nc.gpsimd.collective_compute(
                        "AllGather", mybir.AluOpType.bypass,
                        replica_groups=[[0, 1], [2, 3], [4, 5], [6, 7]],
                        ins=[own_vp_a[:].opt()], outs=[gath_vp_a[:].opt()])
            nc.gpsimd.collective_compute(
                "AllGather", mybir.AluOpType.bypass,
                replica_groups=[[0, 1], [2, 3], [4, 5], [6, 7]],
                ins=[own_vp_b[:].opt()], outs=[gath_vp_b[:].opt()])

            for g in range(4, ND):
                g_unit(g)
            # gathered VP tiles for phase 4, on the gpsimd queue (each group
            # blocked there until its collective completes). gath_a rows =
            # [lo tiles 0-3 | hi tiles 0-3] = global tiles 0-3 and 8-11;
            # gath_b = 4-7 and 12-15 — global order for both pair members.
            vp = [None] * NS
            for t_glob in range(NS):
                gath = gath_vp_a if (t_glob // 4) % 2 == 0 else gath_vp_b
                r = (t_glob // 8) * 4 + (t_glob % 4)
                t = ptile([P, D], f"vp{t_glob}", f"vp{t_glob}")
                nc.gpsimd.dma_start(t[:], gath[r * P:(r + 1) * P, :])
                vp[t_glob] = t

            # ---- phase 2b: scoresT[sk] = X^T[sk-blk].T @ G; exp; acc ----
            expT = []
            for sk in range(NS):
                expT.append(ptile([P, SQ], f"expT{sk}", f"expT{sk}"))
            acc_sb = ptile([P, SQ], "acc_sb", "acc_sb", dt=FP)
            for sk in range(NS):
                psc = qbank(sk % 2, f"psc_{sk}")
                for d in range(ND):
                    xs = xs_t[sk][:, d * P:(d + 1) * P]
                    nc.tensor.matmul(psc[:, 0:512], xs, g_sb[d][:, 0:512],
                                     start=(d == 0), stop=(d == ND - 1))
                    nc.tensor.matmul(psc[:, 512:1024], xs,
                                     g_sb[d][:, 512:1024],
                                     start=(d == 0), stop=(d == ND - 1))
                nc.scalar.activation(expT[sk][:], psc[:], Exp, scale=SCALE)
                if sk == 0:
                    nc.vector.tensor_copy(acc_sb[:], expT[0][:])
                else:
                    nc.vector.tensor_tensor(acc_sb[:], acc_sb[:], expT[sk][:], ADD)

            # ---- rowsum -> per-q-chunk reciprocal columns [128, 8] ----
            pr = qbank(0, "pr", shape=(P, 8))
            for c in range(ND):
                nc.tensor.matmul(pr[:, c:c + 1], acc_sb[:, c * P:(c + 1) * P],
                                 onesc_sb[:], start=True, stop=True)
            recip_sb = ptile([P, 8], "recip", "recip", dt=FP)
            nc.vector.reciprocal(recip_sb[:], pr[:, 0:8])

            # ---- phase 4: y[qc][128, 1024] = sum_sk expT[sk][:,qc]^T @ VP[sk]
            PASSES = [(0, 1), (2, 3), (4, 5), (6,), (7,)]
            for chunks in PASSES:
                pts = [qbank(c % 4, f"py_{c}") for c in chunks]
                for sk in range(NS):
                    for i, c in enumerate(chunks):
                        lt = expT[sk][:, c * P:(c + 1) * P]
                        nc.tensor.matmul(pts[i][:, 0:512], lt,
                                         vp[sk][:, 0:512],
                                         start=(sk == 0), stop=(sk == NS - 1))
                        nc.tensor.matmul(pts[i][:, 512:1024], lt,
                                         vp[sk][:, 512:1024],
                                         start=(sk == 0), stop=(sk == NS - 1))
                for i, c in enumerate(chunks):
                    ysb = ptile([P, D], f"ysb_{c}", f"ysb{c % 2}", dt=FP)
                    for h in range(2):
                        sl = slice(h * 512, (h + 1) * 512)
                        nc.scalar.activation(ysb[:, sl], pts[i][:, sl], Copy,
                                             scale=recip_sb[:, c:c + 1])
                        nc.vector.tensor_tensor(ysb[:, sl], ysb[:, sl],
                                                biasb_sb[:, sl], ADD)
                        nc.sync.dma_start(y_d[c * P:(c + 1) * P, sl], ysb[:, sl])

    nc.compile()
    return nc


def _get_nc():
    if "nc" not in _CACHED:
        _CACHED["nc"] = _build_nc()
    return _CACHED["nc"]


def make_in_maps(x, w_qkv, w_proj, b_proj):
    import ml_dtypes
    BF = ml_dtypes.bfloat16
    wq = w_qkv[0:D]
    wk = w_qkv[D:2 * D]
    wv = w_qkv[2 * D:3 * D]
    w2 = wk.T @ wq                   # scoresT = X W2 Xq^T
    w3 = w_proj @ wv                 # y = attn (X W3^T)
    w2T = np.ascontiguousarray(w2.T).astype(BF)
    w3T = np.ascontiguousarray(w3.T).astype(BF)
    biasb = np.ascontiguousarray(
        np.broadcast_to(b_proj[None, :], (P, D))).astype(np.float32)
    onesc = np.ones((P, 1), dtype=np.float32)
    in_maps = []
    for c in range(8):
        b, h = c // 2, c % 2
        # xtr[sk*128+p, d*128+j] = x[b][sk*128+j, d*128+p]
        xtr = np.ascontiguousarray(
            x[b].reshape(NS, P, ND, P).transpose(0, 3, 2, 1)
            .reshape(S, D)).astype(BF)
        xqt = np.ascontiguousarray(x[b, h * SQ:(h + 1) * SQ].T).astype(BF)
        in_maps.append({
            "xtr": xtr, "xqt": xqt, "w2t": w2T, "w3t": w3T,
            "biasb": biasb, "onesc": onesc,
        })
    return in_maps


def gather_out(results):
    out = np.empty((4, S, D), dtype=np.float32)
    for c in range(8):
        b, h = c // 2, c % 2
        out[b, h * SQ:(h + 1) * SQ] = results[c]["y"]
    return out


def kernel(x, w_qkv, w_proj, b_proj):
    from concourse import bass_utils
    nc = _get_nc()
    in_maps = make_in_maps(np.asarray(x, dtype=np.float32),
                           np.asarray(w_qkv, dtype=np.float32),
                           np.asarray(w_proj, dtype=np.float32),
                           np.asarray(b_proj, dtype=np.float32))
    res = bass_utils.run_bass_kernel_spmd(nc, in_maps, list(range(8))).results
    return gather_out(res)


# revision 6
# speedup vs baseline: 1.0051x; 1.0051x over previous
"""Trainium2 Bass kernel for nn_AttentionBlock (B=4, S=2048, D=1024).

Sharding: 8 cores = 4 batches x 2 query-halves. Each core owns 1024
queries of one batch and produces y[own queries, 1024] directly in
row-major [q, f] layout. All matmuls bf16 (fp32 PSUM accumulate),
rel err ~2e-3 (gate 2e-2). HW exec ~188-190us (vs 292.8us baseline).

Host-side weight folds (x-independent):
    W2 = Wk^T @ Wq  ->  scoresT = X W2 Xq^T   ([key, query] layout)
    W3 = Wp @ Wv    ->  y = attn (X W3^T) = attn VP
Q, K, V, and the projection never exist on the device.

PE stream (~99% dense, 752 x 512-col matmuls ~= 169us):
  1a: G[0:4]  = W2[g,:] @ Xq^T                              (64 MMs)
  2a: VP = X @ W3^T for the core's OWN 1024 keys only      (128 MMs)
      -> DRAM bounce -> two pairwise AllGathers ([[0,1],[2,3],[4,5],
      [6,7]], 1MB each, first triggered after 4 VP tiles) assemble the
      full [2048, 1024] VP on both cores, overlapped with 1b+2b.
      Removes the VP duplication across the batch pair (~28us of PE).
      SPMD rank-symmetry: own keys == own queries (same global half),
      and the gathered buffers are in GLOBAL key order on both pair
      members, so the single shared graph stays rank-agnostic.
  1b: G[4:8]                                                (64 MMs)
  2b: scoresT[sk] = X^T[sk].T @ G; exp on ScalarE -> bf16 expT;
      fp32 acc_sb += expT on DVE                           (256 MMs)
  rowsum: 8 tiny N=1 matmuls (stationary acc chunk, moving ones col)
      -> [128q, 1] per query chunk; one DVE reciprocal [128, 8]
  4:  y[qc] = sum_sk expT[sk][:, qc].T @ VP[sk]            (256 MMs)
      epilogue: ScalarE Copy scaled by the per-partition reciprocal
      column, DVE add of a pre-broadcast bias tile, DMA out.

Scheduling notes (hard-won):
  - The Sync sequencer pushes dma_starts strictly in order at ~0.6us
    each; bulk pushes ahead of a latency-critical descriptor delay it
    by tens of us. So: inputs are coalesced to one push per [128, 1024]
    tile (the xtr host layout gives one push per KEY TILE), and all
    collective-path DMAs + triggers live on the idle GPSIMD queue.
  - The AllGather data phase occupies all 16 DMA engines (~12-45us,
    high variance); every input the compute needs is SBUF-resident
    before it starts, so nothing starves mid-collective.
  - Gather-ins are emitted in phase-4 consumption order (tiles 0-3
    after collective A, 4-7 after B, 8-15 instantly), so pass 0 can
    start while a slow second exchange is still in flight.
"""

import numpy as np
from contextlib import ExitStack

D = 1024
S = 2048
SQ = 1024  # queries per core
P = 128
ND = D // P   # 8
NS = S // P   # 16
SCALE = float(1.0 / np.sqrt(np.float32(D)).astype(np.float32))

_CACHED = {}


def _build_nc():
    import concourse.tile as tile
    from concourse import bacc, mybir

    BF = mybir.dt.bfloat16
    FP = mybir.dt.float32
    Exp = mybir.ActivationFunctionType.Exp
    Copy = mybir.ActivationFunctionType.Copy
    ADD = mybir.AluOpType.add

    nc = bacc.Bacc("TRN2", target_bir_lowering=False, num_devices=8)
    # xtr: host-rearranged X so each key tile sk is one contiguous row
    # block: xtr[sk*128 + p, d*128 + j] = X[sk*128 + j, d*128 + p].
    # One DMA push per key tile (16 total) instead of 128.
    xtr_d = nc.declare_dram_parameter("xtr", [S, D], BF, isOutput=False)
    xqt_d = nc.declare_dram_parameter("xqt", [D, SQ], BF, isOutput=False)
    w2t_d = nc.declare_dram_parameter("w2t", [D, D], BF, isOutput=False)
    w3t_d = nc.declare_dram_parameter("w3t", [D, D], BF, isOutput=False)
    biasb_d = nc.declare_dram_parameter("biasb", [P, D], FP, isOutput=False)
    onesc_d = nc.declare_dram_parameter("onesc", [P, 1], FP, isOutput=False)
    y_d = nc.declare_dram_parameter("y", [SQ, D], FP, isOutput=True)

    with tile.TileContext(nc) as tc:
        with ExitStack() as ctx:
            pool = ctx.enter_context(tc.tile_pool(name="main", bufs=1))
            psum = ctx.enter_context(tc.tile_pool(name="psum", bufs=1, space="PSUM"))
            dram = ctx.enter_context(tc.tile_pool(name="dram", bufs=1, space="DRAM"))

            def ptile(shape, name, tag, bufs=1, dt=BF):
                return pool.tile(shape, dt, name=name, tag=tag, bufs=bufs)

            def qbank(i, name, shape=(P, 1024)):
                return psum.tile(list(shape), FP, name=name, tag=f"q{i}", bufs=1)

            # ---- resident inputs ----
            xq = []
            w2t = []
            for d in range(ND):
                t = ptile([P, SQ], f"xq{d}", f"xq{d}")
                nc.sync.dma_start(t[:], xqt_d[d * P:(d + 1) * P, :])
                xq.append(t)
                t = ptile([P, D], f"w2t{d}", f"w2t{d}")
                nc.sync.dma_start(t[:], w2t_d[d * P:(d + 1) * P, :])
                w2t.append(t)
            w3t = []
            for d in range(ND):
                t = ptile([P, D], f"w3t{d}", f"w3t{d}")
                nc.sync.dma_start(t[:], w3t_d[d * P:(d + 1) * P, :])
                w3t.append(t)
            onesc_sb = ptile([P, 1], "onesc", "onesc", dt=FP)
            nc.sync.dma_start(onesc_sb[:], onesc_d[:, :])
            biasb_sb = ptile([P, D], "biasb", "biasb", dt=FP)
            nc.sync.dma_start(biasb_sb[:], biasb_d[:, :])
            # preload the full X^T key stream: all input DMA lands by ~31us,
            # well before the collective data phase hogs the DMA engines,
            # so the scores units never touch DMA mid-collective.
            xs_t = []
            for sk in range(NS):
                t = ptile([P, D], f"xs_{sk}", f"xs_{sk}")
                nc.sync.dma_start(t[:], xtr_d[sk * P:(sk + 1) * P, :])
                xs_t.append(t)

            # DRAM bounce buffers for the VP exchange, split in two halves:
            # each transfer is 1MB instead of 2MB and the first one starts
            # ~20us earlier, thinning the slow-collective tail
            own_vp_a = dram.tile([SQ // 2, D], BF)
            own_vp_b = dram.tile([SQ // 2, D], BF)
            gath_vp_a = dram.tile([S // 2, D], BF)
            gath_vp_b = dram.tile([S // 2, D], BF)

            # ---- phase 1: G[g][128, SQ] = sum_d W2[g,d] @ Xq^T[d] ----
            g_sb = []
            for g in range(ND):
                g_sb.append(ptile([P, SQ], f"g{g}", f"g{g}"))

            def g_unit(g):
                pg = qbank(g % 4, f"pg_{g}")
                for d in range(ND):
                    lt = w2t[d][:, g * P:(g + 1) * P]
                    nc.tensor.matmul(pg[:, 0:512], lt, xq[d][:, 0:512],
                                     start=(d == 0), stop=(d == ND - 1))
                    nc.tensor.matmul(pg[:, 512:1024], lt, xq[d][:, 512:1024],
                                     start=(d == 0), stop=(d == ND - 1))
                nc.vector.tensor_copy(g_sb[g][:], pg[:])

            # half of phase 1, then the VP units: the first collective
            # triggers ~35us earlier, capping slow-exchange stalls
            for g in range(4):
                g_unit(g)

            # ---- phase 2a: VP[j][128, 1024] = X_own[j-blk] @ W3^T for own
            #      keys only; bounce each tile to DRAM; pair-AllGather.
            #      All collective-path DMAs + the trigger live on the GPSIMD
            #      queue so they are not pushed behind the sync queue's
            #      ~160 input pushes.
            vp_own = []
            for j in range(ND):
                vp_own.append(ptile([P, D], f"vpo{j}", f"vpo{j % 4}", bufs=2))
            for j in range(ND):
                pvp = qbank(2 + j % 2, f"pv_{j}")
                for d in range(ND):
                    # own keys == own queries (same global half), so the
                    # stationary tile is a slice of the resident xq tiles
                    lt = xq[d][:, j * P:(j + 1) * P]
                    nc.tensor.matmul(pvp[:, 0:512], lt, w3t[d][:, 0:512],
                                     start=(d == 0), stop=(d == ND - 1))
                    nc.tensor.matmul(pvp[:, 512:1024], lt,
                                     w3t[d][:, 512:1024],
                                     start=(d == 0), stop=(d == ND - 1))
                nc.vector.tensor_copy(vp_own[j][:], pvp[:])
                half = own_vp_a if j < 4 else own_vp_b
                nc.gpsimd.dma_start(half[(j % 4) * P:(j % 4 + 1) * P, :],
                                    vp_own[j][:])
                if j == 3:
                    nc.gpsimd.collective_compute(
                        "AllGather", mybir.AluOpType.bypass,
                        replica_groups=[[0, 1], [2, 3], [4, 5], [6, 7]],
                        ins=[own_vp_a[:].opt()], outs=[gath_vp_a[:].opt()])
            nc.gpsimd.collective_compute(
                "AllGather", mybir.AluOpType.bypass,
                replica_groups=[[0, 1], [2, 3], [4, 5], [6, 7]],
                ins=[own_vp_b[:].opt()], outs=[gath_vp_b[:].opt()])

            for g in range(4, ND):
                g_unit(g)
            # gathered VP tiles for phase 4, on the gpsimd queue (each group
            # blocked there until its collective completes). gath_a rows =
            # [lo tiles 0-3 | hi tiles 0-3] = global tiles 0-3 and 8-11;
            # gath_b = 4-7 and 12-15 — global order for both pair members.
            vp = [None] * NS
            for t_glob in range(NS):
                gath = gath_vp_a if (t_glob // 4) % 2 == 0 else gath_vp_b
                r = (t_glob // 8) * 4 + (t_glob % 4)
                t = ptile([P, D], f"vp{t_glob}", f"vp{t_glob}")
                nc.gpsimd.dma_start(t[:], gath[r * P:(r + 1) * P, :])
                vp[t_glob] = t

            # ---- phase 2b: scoresT[sk] = X^T[sk-blk].T @ G; exp; acc ----
            expT = []
            for sk in range(NS):
                expT.append(ptile([P, SQ], f"expT{sk}", f"expT{sk}"))
            acc_sb = ptile([P, SQ], "acc_sb", "acc_sb", dt=FP)
            for sk in range(NS):
                psc = qbank(sk % 2, f"psc_{sk}")
                for d in range(ND):
                    xs = xs_t[sk][:, d * P:(d + 1) * P]
                    nc.tensor.matmul(psc[:, 0:512], xs, g_sb[d][:, 0:512],
                                     start=(d == 0), stop=(d == ND - 1))
                    nc.tensor.matmul(psc[:, 512:1024], xs,
                                     g_sb[d][:, 512:1024],
                                     start=(d == 0), stop=(d == ND - 1))
                nc.scalar.activation(expT[sk][:], psc[:], Exp, scale=SCALE)
                if sk == 0:
                    nc.vector.tensor_copy(acc_sb[:], expT[0][:])
                else:
                    nc.vector.tensor_tensor(acc_sb[:], acc_sb[:], expT[sk][:], ADD)

            # ---- rowsum -> per-q-chunk reciprocal columns [128, 8] ----
            pr = qbank(0, "pr", shape=(P, 8))
            for c in range(ND):
                nc.tensor.matmul(pr[:, c:c + 1], acc_sb[:, c * P:(c + 1) * P],
                                 onesc_sb[:], start=True, stop=True)
            recip_sb = ptile([P, 8], "recip", "recip", dt=FP)
            nc.vector.reciprocal(recip_sb[:], pr[:, 0:8])

            # ---- phase 4: y[qc][128, 1024] = sum_sk expT[sk][:,qc]^T @ VP[sk]
            PASSES = [(0, 1), (2, 3), (4, 5), (6,)]
            for chunks in PASSES:
                pts = [qbank(c % 4, f"py_{c}") for c in chunks]
                for sk in range(NS):
                    for i, c in enumerate(chunks):
                        lt = expT[sk][:, c * P:(c + 1) * P]
                        nc.tensor.matmul(pts[i][:, 0:512], lt,
                                         vp[sk][:, 0:512],
                                         start=(sk == 0), stop=(sk == NS - 1))
                        nc.tensor.matmul(pts[i][:, 512:1024], lt,
                                         vp[sk][:, 512:1024],
                                         start=(sk == 0), stop=(sk == NS - 1))
                for i, c in enumerate(chunks):
                    ysb = ptile([P, D], f"ysb_{c}", f"ysb{c % 2}", dt=FP)
                    for h in range(2):
                        sl = slice(h * 512, (h + 1) * 512)
                        nc.scalar.activation(ysb[:, sl], pts[i][:, sl], Copy,
                                             scale=recip_sb[:, c:c + 1])
                        nc.vector.tensor_tensor(ysb[:, sl], ysb[:, sl],
                                                biasb_sb[:, sl], ADD)
                        nc.sync.dma_start(y_d[c * P:(c + 1) * P, sl], ysb[:, sl])

            # final chunk: half-major sk sweeps so half 0's epilogue + DMA
            # overlap half 1's accumulation; pushes on the idle gpsimd queue
            # (the sync sequencer's ~0.6us/push would gate the tail)
            pt7 = qbank(3, "py_7")
            ysb7 = ptile([P, D], "ysb_7", "ysb1", dt=FP)
            for h in range(2):
                sl = slice(h * 512, (h + 1) * 512)
                for sk in range(NS):
                    nc.tensor.matmul(pt7[:, sl], expT[sk][:, 7 * P:8 * P],
                                     vp[sk][:, sl],
                                     start=(sk == 0), stop=(sk == NS - 1))
                nc.scalar.activation(ysb7[:, sl], pt7[:, sl], Copy,
                                     scale=recip_sb[:, 7:8])
                nc.vector.tensor_tensor(ysb7[:, sl], ysb7[:, sl],
                                        biasb_sb[:, sl], ADD)
                nc.gpsimd.dma_start(y_d[7 * P:8 * P, sl], ysb7[:, sl])

    nc.compile()
    return nc


def _get_nc():
    if "nc" not in _CACHED:
        _CACHED["nc"] = _build_nc()
    return _CACHED["nc"]


def make_in_maps(x, w_qkv, w_proj, b_proj):
    import ml_dtypes
    BF = ml_dtypes.bfloat16
    wq = w_qkv[0:D]
    wk = w_qkv[D:2 * D]
    wv = w_qkv[2 * D:3 * D]
    w2 = wk.T @ wq                   # scoresT = X W2 Xq^T
    w3 = w_proj @ wv                 # y = attn (X W3^T)
    w2T = np.ascontiguousarray(w2.T).astype(BF)
    w3T = np.ascontiguousarray(w3.T).astype(BF)
    biasb = np.ascontiguousarray(
        np.broadcast_to(b_proj[None, :], (P, D))).astype(np.float32)
    onesc = np.ones((P, 1), dtype=np.float32)
    in_maps = []
    for c in range(8):
        b, h = c // 2, c % 2
        # xtr[sk*128+p, d*128+j] = x[b][sk*128+j, d*128+p]
        xtr = np.ascontiguousarray(
            x[b].reshape(NS, P, ND, P).transpose(0, 3, 2, 1)
            .reshape(S, D)).astype(BF)
        xqt = np.ascontiguousarray(x[b, h * SQ:(h + 1) * SQ].T).astype(BF)
        in_maps.append({
            "xtr": xtr, "xqt": xqt, "w2t": w2T, "w3t": w3T,
            "biasb": biasb, "onesc": onesc,
        })
    return in_maps


def gather_out(results):
    out = np.empty((4, S, D), dtype=np.float32)
    for c in range(8):
        b, h = c // 2, c % 2
        out[b, h * SQ:(h + 1) * SQ] = results[c]["y"]
    return out


def kernel(x, w_qkv, w_proj, b_proj):
    from concourse import bass_utils
    nc = _get_nc()
    in_maps = make_in_maps(np.asarray(x, dtype=np.float32),
                           np.asarray(w_qkv, dtype=np.float32),
                           np.asarray(w_proj, dtype=np.float32),
                           np.asarray(b_proj, dtype=np.float32))
    res = bass_utils.run_bass_kernel_spmd(nc, in_maps, list(range(8))).results
    return gather_out(res)
